# revision 15
# baseline (speedup 1.0000x reference)
"""MoE MLP (64 routed experts, top-6, shared expert) on 8 Trainium2 NeuronCores.

Strategy (per sharding hint):
  Phase 1 (data-parallel over tokens, 512 tok/core): gate GEMM in fp32,
    softmax + top-6 mask on device.
  Host: dispatch bookkeeping (the "all-to-all"): sort pairs by expert,
    serpentine-assign experts to cores/slots, build padded token buffers.
  Phase 2 (expert-parallel, 8 routed experts + the shared expert per core):
    grouped GEMMs in bf16 (fp32 accumulate), gelu, bias, per-token
    gate-score scaling on device. Slot widths are baked per call from the
    actual per-expert token counts, so padding waste is minimal.
  Host: combine (scatter-add routed, place shared), balance-loss terms.
"""

import sys

if "/opt/trn_rl_repo" not in sys.path:
    sys.path.insert(0, "/opt/trn_rl_repo")

import numpy as np
import ml_dtypes

import concourse.bass as bass  # noqa: F401
from concourse import bacc, mybir
from concourse.masks import make_identity
from concourse.bass import ts
from concourse.tile import TileContext
from concourse.bass_utils import run_bass_kernel_spmd

F32 = mybir.dt.float32
BF16 = mybir.dt.bfloat16
BF16_NP = ml_dtypes.bfloat16

E = 64          # routed experts
K = 6           # top-k
D = 768         # model dim
Hd = 1536       # expert hidden dim
B, N = 2, 2048
T = B * N       # 4096 tokens
NCORES = 8
TPC = T // NCORES       # 512 tokens/core in phase 1
EPC = E // NCORES       # 8 routed experts/core in phase 2
NSLOT = EPC + 1         # + shared-expert slot
KD = D // 128           # 6 contraction tiles over D
KH = Hd // 128          # 12 contraction tiles over H
MT = TPC // 128         # 4 gate token tiles per core


def _bass():
    return bacc.Bacc("TRN2", target_bir_lowering=False, debug=False,
                     num_devices=NCORES)


# --------------------------------------------------------------------------
# Phase 1: gate GEMM (fp32) + softmax + top-6 mask + prob/count column sums
# --------------------------------------------------------------------------
def build_phase1():
    nc = _bass()
    xT32 = nc.dram_tensor("xT32", [D, TPC], F32, kind="ExternalInput")
    gwT = nc.dram_tensor("gwT", [D, E], F32, kind="ExternalInput")
    sel = nc.dram_tensor("sel", [TPC, E], F32, kind="ExternalOutput")
    colsums = nc.dram_tensor("colsums", [1, 2 * E], F32, kind="ExternalOutput")

    with TileContext(nc) as tc:
        with (
            tc.tile_pool(name="weights", bufs=1) as wpool,
            tc.tile_pool(name="work", bufs=2) as pool,
            tc.tile_pool(name="gpsum", bufs=2, space="PSUM") as gpsum,
            tc.tile_pool(name="cpsum", bufs=1, space="PSUM") as cpsum,
        ):
            gw = wpool.tile([128, KD, E], F32)
            nc.sync.dma_start(gw[:], gwT.ap().rearrange("(k p) e -> p k e", p=128))
            x32 = wpool.tile([128, KD, TPC], F32)
            nc.sync.dma_start(x32[:], xT32.ap().rearrange("(k p) t -> p k t", p=128))

            ones = wpool.tile([128, 1], F32)
            nc.vector.memset(ones[:], 1.0)
            idt = wpool.tile([E, E], F32)
            make_identity(nc, idt[:])

            # logits^T [E, TPC] with gate_w stationary (6 big fp32 matmuls),
            # then PE-transpose each 128-token tile back to [tok, E]
            pl = cpsum.tile([E, TPC], F32, tag="ltpsum")
            for k in range(KD):
                nc.tensor.matmul(pl[:], gw[:, k, :], x32[:, k, :],
                                 start=(k == 0), stop=(k == KD - 1))
            lsb = wpool.tile([E, TPC], F32)
            nc.vector.tensor_copy(lsb[:], pl[:])

            probs_all = wpool.tile([128, MT, E], F32)
            mask_all = wpool.tile([128, MT, E], F32)
            pcp = cpsum.tile([1, E], F32, tag="pcp")
            pcm = cpsum.tile([1, E], F32, tag="pcm")
            for m in range(MT):
                pg = gpsum.tile([128, E], F32, tag="gatepsum")
                nc.tensor.transpose(pg[:], lsb[:, ts(m, 128)], idt[:])
                nmax = pool.tile([128, 1], F32, tag="nmax")
                nc.vector.reduce_max(nmax[:], pg[:], axis=mybir.AxisListType.X,
                                     negate=True)
                exps = pool.tile([128, E], F32, tag="exps")
                rsum = pool.tile([128, 1], F32, tag="rsum")
                nc.scalar.activation(exps[:], pg[:],
                                     mybir.ActivationFunctionType.Exp,
                                     bias=nmax[:, 0:1], scale=1.0,
                                     accum_out=rsum[:])
                rinv = pool.tile([128, 1], F32, tag="rinv")
                nc.vector.reciprocal(rinv[:], rsum[:])
                probs = probs_all[:, m, :]
                nc.vector.tensor_scalar_mul(probs, exps[:], rinv[:, 0:1])
                m8 = pool.tile([128, 8], F32, tag="m8")
                nc.vector.max(m8[:], probs)
                mask = mask_all[:, m, :]
                nc.vector.tensor_scalar(mask, probs, m8[:, 5:6], None,
                                        op0=mybir.AluOpType.is_ge)
                selt = pool.tile([128, E], F32, tag="selt")
                nc.vector.tensor_mul(selt[:], probs, mask)
                nc.sync.dma_start(sel.ap()[ts(m, 128), :], selt[:])
                nc.tensor.matmul(pcp[:], ones[:], probs,
                                 start=(m == 0), stop=(m == MT - 1))
                nc.tensor.matmul(pcm[:], ones[:], mask,
                                 start=(m == 0), stop=(m == MT - 1))
            csum = pool.tile([1, 2 * E], F32, tag="csum")
            nc.vector.tensor_copy(csum[:, 0:E], pcp[:])
            nc.vector.tensor_copy(csum[:, E:2 * E], pcm[:])
            nc.sync.dma_start(colsums.ap()[:, :], csum[:])

    nc.compile()
    return nc


# --------------------------------------------------------------------------
# Phase 2: NSLOT expert slots per core with static widths (slot 0 = shared)
# --------------------------------------------------------------------------
def build_phase2(widths):
    SLOTS = int(sum(widths))
    offs = np.concatenate([[0], np.cumsum(widths)]).astype(int)
    nc = _bass()
    xdT = nc.dram_tensor("xdT", [D, SLOTS], BF16, kind="ExternalInput")
    # w1T[e, h] is one contiguous [128, KD*128] lhsT block for h-tile h
    w1T = nc.dram_tensor("w1T", [NSLOT, KH, 128, KD * 128], BF16,
                         kind="ExternalInput")
    w2T = nc.dram_tensor("w2T", [NSLOT, KD, 128, KH * 128], BF16,
                         kind="ExternalInput")
    b1 = nc.dram_tensor("b1", [NSLOT, 128, KH], F32, kind="ExternalInput")
    b2 = nc.dram_tensor("b2", [NSLOT, 128, KD], F32, kind="ExternalInput")
    scores = nc.dram_tensor("scores", [1, SLOTS], F32, kind="ExternalInput")
    ydT = nc.dram_tensor("ydT", [D, SLOTS], F32, kind="ExternalOutput")

    xdT_r = xdT.ap().rearrange("(k p) t -> p k t", p=128)
    ydT_r = ydT.ap().rearrange("(k p) t -> p k t", p=128)

    with TileContext(nc) as tc:
        with (
            tc.tile_pool(name="wts", bufs=2) as wpool,
            tc.tile_pool(name="work", bufs=3) as pool,
            tc.tile_pool(name="ypool", bufs=2) as ypool,
            tc.tile_pool(name="hpsum", bufs=3, space="PSUM") as hpsum,
            tc.tile_pool(name="ypsum", bufs=3, space="PSUM") as ypsum,
        ):
            for e in range(NSLOT):
                W = int(widths[e])
                # DMA order: first matmul's deps first (first w1 h-tile +
                # first x tile + gelu bias), then the rest of the weights.
                w1 = wpool.tile([128, KH, KD * 128], BF16, tag="w1")
                nc.sync.dma_start(w1[:, 0, :], w1T.ap()[e, 0])
                b1e = wpool.tile([128, KH], F32, tag="b1")
                nc.sync.dma_start(b1e[:], b1.ap()[e])
                tn0 = min(512, W)
                xe0 = pool.tile([128, KD, tn0], BF16, tag="xe")
                nc.sync.dma_start(xe0[:], xdT_r[:, :, offs[e]:offs[e] + tn0])
                sc0 = pool.tile([128, tn0], F32, tag="sc")
                nc.sync.dma_start(
                    sc0[:], scores.ap()[:, offs[e]:offs[e] + tn0]
                    .to_broadcast((128, tn0)))
                for h in range(1, KH):
                    nc.sync.dma_start(w1[:, h, :], w1T.ap()[e, h])
                w2 = wpool.tile([128, KD, KH * 128], BF16, tag="w2")
                for d in range(KD):
                    nc.sync.dma_start(w2[:, d, :], w2T.ap()[e, d])
                b2e = wpool.tile([128, KD], F32, tag="b2")
                nc.sync.dma_start(b2e[:], b2.ap()[e])

                toff = 0
                while toff < W:
                    tn = min(512, W - toff)
                    base = offs[e] + toff
                    first = toff == 0
                    toff += tn
                    if first:
                        xe, sc = xe0, sc0
                    else:
                        xe = pool.tile([128, KD, tn], BF16, tag="xe")
                        nc.sync.dma_start(xe[:], xdT_r[:, :, base:base + tn])
                        sc = pool.tile([128, tn], F32, tag="sc")
                        nc.sync.dma_start(
                            sc[:], scores.ap()[:, base:base + tn]
                            .to_broadcast((128, tn)))

                    hsb = pool.tile([128, KH, tn], BF16, tag="hsb")
                    for h in range(KH):
                        ph = hpsum.tile([128, tn], F32, tag="hpsum")
                        for k in range(KD):
                            nc.tensor.matmul(ph[:], w1[:, h, ts(k, 128)],
                                             xe[:, k, :],
                                             start=(k == 0), stop=(k == KD - 1))
                        nc.scalar.activation(hsb[:, h, :], ph[:],
                                             mybir.ActivationFunctionType.Gelu,
                                             bias=b1e[:, h:h + 1], scale=1.0)
                    for d in range(KD):
                        py = ypsum.tile([128, tn], F32, tag="ypsum")
                        for h in range(KH):
                            nc.tensor.matmul(py[:], w2[:, d, ts(h, 128)],
                                             hsb[:, h, :],
                                             start=(h == 0), stop=(h == KH - 1))
                        yb = ypool.tile([128, tn], F32, tag="yb")
                        nc.vector.tensor_scalar_add(yb[:], py[:], b2e[:, d:d + 1])
                        ym = ypool.tile([128, tn], F32, tag="ym")
                        nc.vector.tensor_mul(ym[:], yb[:], sc[:])
                        nc.sync.dma_start(ydT_r[:, d, base:base + tn], ym[:])

    nc.compile()
    return nc


# --------------------------------------------------------------------------
# Host orchestration
# --------------------------------------------------------------------------
_CACHE = {}


def _get_phase1():
    if "p1" not in _CACHE:
        _CACHE["p1"] = build_phase1()
    return _CACHE["p1"]


def _get_phase2(widths):
    key = ("p2", tuple(int(w) for w in widths))
    if key not in _CACHE:
        _CACHE[key] = build_phase2(widths)
    return _CACHE[key]


def _run(nc, in_maps, **kw):
    return run_bass_kernel_spmd(nc, in_maps, list(range(NCORES)), **kw).results


def _fix_ties(sel_all):
    """Rows where >K experts tied through the >= threshold (exact fp ties):
    keep the K largest, lower expert index first (jax.lax.top_k order)."""
    cnt = (sel_all > 0).sum(1)
    bad = np.nonzero(cnt != K)[0]
    for r in bad:
        row = sel_all[r]
        nz = np.nonzero(row > 0)[0]
        if len(nz) < K:
            raise RuntimeError(f"token {r}: only {len(nz)} experts selected")
        orderv = np.lexsort((nz, -row[nz]))
        keep = nz[orderv[:K]]
        newrow = np.zeros_like(row)
        newrow[keep] = row[keep]
        sel_all[r] = newrow
    return sel_all


def kernel(**inputs):
    inputs = {k: np.asarray(v) for k, v in inputs.items()}
    x = inputs["x"]
    gate_w = inputs["gate_w"]
    sh1_w, sh1_b = inputs["sh1_w"], inputs["sh1_b"]
    sh2_w, sh2_b = inputs["sh2_w"], inputs["sh2_b"]
    e1_w, e1_b = inputs["e1_w"], inputs["e1_b"]
    e2_w, e2_b = inputs["e2_w"], inputs["e2_b"]

    xf = np.ascontiguousarray(x.reshape(T, D).astype(np.float32))
    xT = np.ascontiguousarray(xf.T)              # (D, T) fp32
    xT16 = xT.astype(BF16_NP)
    gwT = np.ascontiguousarray(gate_w.astype(np.float32).T)

    # ---- phase 1: gate ----
    maps1 = []
    for c in range(NCORES):
        sl = slice(c * TPC, (c + 1) * TPC)
        maps1.append({"xT32": np.ascontiguousarray(xT[:, sl]), "gwT": gwT})
    res1 = _run(_get_phase1(), maps1)

    sel_all = np.concatenate([res1[c]["sel"] for c in range(NCORES)], axis=0)
    sel_all = _fix_ties(sel_all)
    prob_colsum = np.sum(
        [res1[c]["colsums"][0, :E] for c in range(NCORES)], axis=0)

    # ---- host routing ----
    tok_idx, exp_idx = np.nonzero(sel_all > 0)   # token-major
    order = np.argsort(exp_idx, kind="stable")
    s_tok = tok_idx[order]
    s_exp = exp_idx[order]
    s_sc = sel_all[tok_idx, exp_idx][order].astype(np.float32)
    counts = np.bincount(s_exp, minlength=E)
    offs_e = np.concatenate([[0], np.cumsum(counts)])
    pos = np.arange(len(s_exp)) - offs_e[s_exp]

    # serpentine assignment of experts (sorted by count desc) to cores/slots
    sort_e = np.argsort(-counts, kind="stable")
    core_of_e = np.empty(E, np.int64)
    slot_of_e = np.empty(E, np.int64)
    for i, e in enumerate(sort_e):
        rnd = i // NCORES
        j = i % NCORES
        core_of_e[e] = j if rnd % 2 == 0 else NCORES - 1 - j
        slot_of_e[e] = 1 + rnd                   # slot 0 is the shared expert
    # static slot widths: max count over the cores at each slot, 16-aligned
    widths = np.zeros(NSLOT, np.int64)
    widths[0] = TPC
    for s in range(1, NSLOT):
        es = np.nonzero(slot_of_e == s)[0]
        widths[s] = max(16, -(-int(counts[es].max()) // 8) * 8)
    offs_s = np.concatenate([[0], np.cumsum(widths)]).astype(int)
    SLOTS = int(offs_s[-1])

    core_of = core_of_e[s_exp]
    local_slot = offs_s[slot_of_e[s_exp]] + pos

    # per-slot weights: slot 0 shared, slots 1.. routed by assignment
    f32 = np.float32
    def lay_w1(w):          # (Hd, D) -> [KH, 128, KD*128] lhsT blocks
        a = w.reshape(KH, 128, KD, 128).transpose(0, 3, 2, 1)
        return np.ascontiguousarray(a).reshape(KH, 128, KD * 128)

    def lay_w2(w):          # (D, Hd) -> [KD, 128, KH*128] lhsT blocks
        a = w.reshape(KD, 128, KH, 128).transpose(0, 3, 2, 1)
        return np.ascontiguousarray(a).reshape(KD, 128, KH * 128)

    w1_all = np.empty((NSLOT, NCORES, KH, 128, KD * 128), BF16_NP)
    w2_all = np.empty((NSLOT, NCORES, KD, 128, KH * 128), BF16_NP)
    b1_all = np.empty((NSLOT, NCORES, 128, KH), f32)
    b2_all = np.empty((NSLOT, NCORES, 128, KD), f32)
    w1_all[0] = lay_w1(sh1_w.astype(BF16_NP))
    w2_all[0] = lay_w2(sh2_w.astype(BF16_NP))
    b1_all[0] = sh1_b.astype(f32).reshape(KH, 128).T
    b2_all[0] = sh2_b.astype(f32).reshape(KD, 128).T
    for e in range(E):
        c, s = core_of_e[e], slot_of_e[e]
        w1_all[s, c] = lay_w1(e1_w[e].astype(BF16_NP))
        w2_all[s, c] = lay_w2(e2_w[e].astype(BF16_NP))
        b1_all[s, c] = e1_b[e].astype(f32).reshape(KH, 128).T
        b2_all[s, c] = e2_b[e].astype(f32).reshape(KD, 128).T

    maps2 = []
    for c in range(NCORES):
        mcore = core_of == c
        tokmap = np.zeros(SLOTS, np.int64)
        scr = np.zeros(SLOTS, np.float32)
        tokmap[local_slot[mcore]] = s_tok[mcore]
        scr[local_slot[mcore]] = s_sc[mcore]
        # shared slot: this core's own tokens, score 1
        tokmap[0:TPC] = np.arange(c * TPC, (c + 1) * TPC)
        scr[0:TPC] = 1.0
        maps2.append({
            "xdT": np.ascontiguousarray(xT16[:, tokmap]),
            "w1T": np.ascontiguousarray(w1_all[:, c]),
            "w2T": np.ascontiguousarray(w2_all[:, c]),
            "b1": np.ascontiguousarray(b1_all[:, c]),
            "b2": np.ascontiguousarray(b2_all[:, c]),
            "scores": scr[None, :],
        })

    res2 = _run(_get_phase2(widths), maps2)

    # ---- combine ----
    yd_all = np.concatenate(
        [res2[c]["ydT"].T for c in range(NCORES)], axis=0)  # (8*SLOTS, D)
    shared_out = np.concatenate(
        [yd_all[c * SLOTS:c * SLOTS + TPC] for c in range(NCORES)], axis=0)
    gslot = core_of * SLOTS + local_slot                    # expert-major
    inv_order = np.argsort(order, kind="stable")            # token-major view
    vals = yd_all[gslot[inv_order]]
    routed = vals.reshape(T, K, D).sum(axis=1, dtype=np.float32)

    out = (shared_out + routed).reshape(B, N, D).astype(np.float32)
    expert_prob = (prob_colsum / np.float32(T)).astype(np.float32)
    expert_freq = (counts / np.float32(T * K)).astype(np.float32)
    balance_loss = np.float32(0.01) * np.sum(
        expert_freq * expert_prob, dtype=np.float32)
    return out, np.float32(balance_loss), expert_freq, expert_prob


# revision 16
# speedup vs baseline: 1.0425x; 1.0425x over previous
"""MoE MLP (64 routed experts, top-6, shared expert) on 8 Trainium2 NeuronCores.

Strategy (per sharding hint):
  Phase 1 (data-parallel over tokens, 512 tok/core): gate GEMM in fp32,
    softmax + top-6 mask on device.
  Host: dispatch bookkeeping (the "all-to-all"): sort pairs by expert,
    serpentine-assign experts to cores/slots, build padded token buffers.
  Phase 2 (expert-parallel, 8 routed experts + the shared expert per core):
    grouped GEMMs in bf16 (fp32 accumulate), gelu, bias, per-token
    gate-score scaling on device. Slot widths are baked per call from the
    actual per-expert token counts, so padding waste is minimal.
  Host: combine (scatter-add routed, place shared), balance-loss terms.
"""

import sys

if "/opt/trn_rl_repo" not in sys.path:
    sys.path.insert(0, "/opt/trn_rl_repo")

import numpy as np
import ml_dtypes

import concourse.bass as bass  # noqa: F401
from concourse import bacc, mybir
from concourse.masks import make_identity
from concourse.bass import ts
from concourse.tile import TileContext
from concourse.bass_utils import run_bass_kernel_spmd

F32 = mybir.dt.float32
BF16 = mybir.dt.bfloat16
BF16_NP = ml_dtypes.bfloat16

E = 64          # routed experts
K = 6           # top-k
D = 768         # model dim
Hd = 1536       # expert hidden dim
B, N = 2, 2048
T = B * N       # 4096 tokens
NCORES = 8
TPC = T // NCORES       # 512 tokens/core in phase 1
EPC = E // NCORES       # 8 routed experts/core in phase 2
NSLOT = EPC + 1         # + shared-expert slot
KD = D // 128           # 6 contraction tiles over D
KH = Hd // 128          # 12 contraction tiles over H
MT = TPC // 128         # 4 gate token tiles per core


def _bass():
    return bacc.Bacc("TRN2", target_bir_lowering=False, debug=False,
                     num_devices=NCORES)


# --------------------------------------------------------------------------
# Phase 1: gate GEMM (fp32) + softmax + top-6 mask + prob/count column sums
# --------------------------------------------------------------------------
def build_phase1():
    nc = _bass()
    # x pre-chunked by gate token tile: [MT][128 d][KD][128 tok]
    xg = nc.dram_tensor("xg", [MT, 128, KD, 128], F32, kind="ExternalInput")
    gwT = nc.dram_tensor("gwT", [D, E], F32, kind="ExternalInput")
    sel = nc.dram_tensor("sel", [TPC, E], F32, kind="ExternalOutput")
    colsums = nc.dram_tensor("colsums", [1, 2 * E], F32, kind="ExternalOutput")

    with TileContext(nc) as tc:
        with (
            tc.tile_pool(name="weights", bufs=1) as wpool,
            tc.tile_pool(name="work", bufs=2) as pool,
            tc.tile_pool(name="gpsum", bufs=2, space="PSUM") as gpsum,
            tc.tile_pool(name="cpsum", bufs=1, space="PSUM") as cpsum,
        ):
            gw = wpool.tile([128, KD, E], F32)
            nc.sync.dma_start(gw[:], gwT.ap().rearrange("(k p) e -> p k e", p=128))
            xs = []
            for m in range(MT):
                xm = wpool.tile([128, KD, 128], F32, tag=f"xg{m}")
                # host pre-lays each block as [128 d][KD][128 tok]
                nc.sync.dma_start(xm[:], xg.ap()[m])
                xs.append(xm)

            ones = wpool.tile([128, 1], F32)
            nc.vector.memset(ones[:], 1.0)

            probs_all = wpool.tile([128, MT, E], F32)
            mask_all = wpool.tile([128, MT, E], F32)
            pcp = cpsum.tile([1, E], F32, tag="pcp")
            pcm = cpsum.tile([1, E], F32, tag="pcm")
            for m in range(MT):
                pg = gpsum.tile([128, E], F32, tag="gatepsum")
                for k in range(KD):
                    nc.tensor.matmul(pg[:], xs[m][:, k, :], gw[:, k, :],
                                     start=(k == 0), stop=(k == KD - 1))
                nmax = pool.tile([128, 1], F32, tag="nmax")
                nc.vector.reduce_max(nmax[:], pg[:], axis=mybir.AxisListType.X,
                                     negate=True)
                exps = pool.tile([128, E], F32, tag="exps")
                rsum = pool.tile([128, 1], F32, tag="rsum")
                nc.scalar.activation(exps[:], pg[:],
                                     mybir.ActivationFunctionType.Exp,
                                     bias=nmax[:, 0:1], scale=1.0,
                                     accum_out=rsum[:])
                rinv = pool.tile([128, 1], F32, tag="rinv")
                nc.vector.reciprocal(rinv[:], rsum[:])
                probs = probs_all[:, m, :]
                nc.vector.tensor_scalar_mul(probs, exps[:], rinv[:, 0:1])
                m8 = pool.tile([128, 8], F32, tag="m8")
                nc.vector.max(m8[:], probs)
                mask = mask_all[:, m, :]
                nc.vector.tensor_scalar(mask, probs, m8[:, 5:6], None,
                                        op0=mybir.AluOpType.is_ge)
                selt = pool.tile([128, E], F32, tag="selt")
                nc.vector.tensor_mul(selt[:], probs, mask)
                nc.sync.dma_start(sel.ap()[ts(m, 128), :], selt[:])
                nc.tensor.matmul(pcp[:], ones[:], probs,
                                 start=(m == 0), stop=(m == MT - 1))
                nc.tensor.matmul(pcm[:], ones[:], mask,
                                 start=(m == 0), stop=(m == MT - 1))
            csum = pool.tile([1, 2 * E], F32, tag="csum")
            nc.vector.tensor_copy(csum[:, 0:E], pcp[:])
            nc.vector.tensor_copy(csum[:, E:2 * E], pcm[:])
            nc.sync.dma_start(colsums.ap()[:, :], csum[:])

    nc.compile()
    return nc


# --------------------------------------------------------------------------
# Phase 2: NSLOT expert slots per core with static widths (slot 0 = shared)
# --------------------------------------------------------------------------
def build_phase2(widths):
    SLOTS = int(sum(widths))
    offs = np.concatenate([[0], np.cumsum(widths)]).astype(int)
    nc = _bass()
    xdT = nc.dram_tensor("xdT", [D, SLOTS], BF16, kind="ExternalInput")
    # w1T[e, h] is one contiguous [128, KD*128] lhsT block for h-tile h
    w1T = nc.dram_tensor("w1T", [NSLOT, KH, 128, KD * 128], BF16,
                         kind="ExternalInput")
    w2T = nc.dram_tensor("w2T", [NSLOT, KD, 128, KH * 128], BF16,
                         kind="ExternalInput")
    b1 = nc.dram_tensor("b1", [NSLOT, 128, KH], F32, kind="ExternalInput")
    b2 = nc.dram_tensor("b2", [NSLOT, 128, KD], F32, kind="ExternalInput")
    scores = nc.dram_tensor("scores", [1, SLOTS], F32, kind="ExternalInput")
    ydT = nc.dram_tensor("ydT", [D, SLOTS], F32, kind="ExternalOutput")

    xdT_r = xdT.ap().rearrange("(k p) t -> p k t", p=128)
    ydT_r = ydT.ap().rearrange("(k p) t -> p k t", p=128)

    with TileContext(nc) as tc:
        with (
            tc.tile_pool(name="wts", bufs=2) as wpool,
            tc.tile_pool(name="work", bufs=3) as pool,
            tc.tile_pool(name="ypool", bufs=2) as ypool,
            tc.tile_pool(name="hpsum", bufs=3, space="PSUM") as hpsum,
            tc.tile_pool(name="ypsum", bufs=3, space="PSUM") as ypsum,
        ):
            for e in range(NSLOT):
                W = int(widths[e])
                # DMA order: first matmul's deps first (first w1 h-tile +
                # first x tile + gelu bias), then the rest of the weights.
                w1 = wpool.tile([128, KH, KD * 128], BF16, tag="w1")
                nc.sync.dma_start(w1[:, 0, :], w1T.ap()[e, 0])
                b1e = wpool.tile([128, KH], F32, tag="b1")
                nc.sync.dma_start(b1e[:], b1.ap()[e])
                tn0 = min(512, W)
                xe0 = pool.tile([128, KD, tn0], BF16, tag="xe")
                nc.sync.dma_start(xe0[:], xdT_r[:, :, offs[e]:offs[e] + tn0])
                sc0 = pool.tile([128, tn0], F32, tag="sc")
                nc.sync.dma_start(
                    sc0[:], scores.ap()[:, offs[e]:offs[e] + tn0]
                    .to_broadcast((128, tn0)))
                for h in range(1, KH):
                    nc.sync.dma_start(w1[:, h, :], w1T.ap()[e, h])
                w2 = wpool.tile([128, KD, KH * 128], BF16, tag="w2")
                for d in range(KD):
                    nc.sync.dma_start(w2[:, d, :], w2T.ap()[e, d])
                b2e = wpool.tile([128, KD], F32, tag="b2")
                nc.sync.dma_start(b2e[:], b2.ap()[e])

                toff = 0
                while toff < W:
                    tn = min(512, W - toff)
                    base = offs[e] + toff
                    first = toff == 0
                    toff += tn
                    if first:
                        xe, sc = xe0, sc0
                    else:
                        xe = pool.tile([128, KD, tn], BF16, tag="xe")
                        nc.sync.dma_start(xe[:], xdT_r[:, :, base:base + tn])
                        sc = pool.tile([128, tn], F32, tag="sc")
                        nc.sync.dma_start(
                            sc[:], scores.ap()[:, base:base + tn]
                            .to_broadcast((128, tn)))

                    hsb = pool.tile([128, KH, tn], BF16, tag="hsb")
                    for h in range(KH):
                        ph = hpsum.tile([128, tn], F32, tag="hpsum")
                        for k in range(KD):
                            nc.tensor.matmul(ph[:], w1[:, h, ts(k, 128)],
                                             xe[:, k, :],
                                             start=(k == 0), stop=(k == KD - 1))
                        nc.scalar.activation(hsb[:, h, :], ph[:],
                                             mybir.ActivationFunctionType.Gelu,
                                             bias=b1e[:, h:h + 1], scale=1.0)
                    for d in range(KD):
                        py = ypsum.tile([128, tn], F32, tag="ypsum")
                        for h in range(KH):
                            nc.tensor.matmul(py[:], w2[:, d, ts(h, 128)],
                                             hsb[:, h, :],
                                             start=(h == 0), stop=(h == KH - 1))
                        yb = ypool.tile([128, tn], F32, tag="yb")
                        nc.vector.tensor_scalar_add(yb[:], py[:], b2e[:, d:d + 1])
                        ym = ypool.tile([128, tn], F32, tag="ym")
                        nc.vector.tensor_mul(ym[:], yb[:], sc[:])
                        nc.sync.dma_start(ydT_r[:, d, base:base + tn], ym[:])

    nc.compile()
    return nc


# --------------------------------------------------------------------------
# Host orchestration
# --------------------------------------------------------------------------
_CACHE = {}


def _get_phase1():
    if "p1" not in _CACHE:
        _CACHE["p1"] = build_phase1()
    return _CACHE["p1"]


def _get_phase2(widths):
    key = ("p2", tuple(int(w) for w in widths))
    if key not in _CACHE:
        _CACHE[key] = build_phase2(widths)
    return _CACHE[key]


def _run(nc, in_maps, **kw):
    return run_bass_kernel_spmd(nc, in_maps, list(range(NCORES)), **kw).results


def _fix_ties(sel_all):
    """Rows where >K experts tied through the >= threshold (exact fp ties):
    keep the K largest, lower expert index first (jax.lax.top_k order)."""
    cnt = (sel_all > 0).sum(1)
    bad = np.nonzero(cnt != K)[0]
    for r in bad:
        row = sel_all[r]
        nz = np.nonzero(row > 0)[0]
        if len(nz) < K:
            raise RuntimeError(f"token {r}: only {len(nz)} experts selected")
        orderv = np.lexsort((nz, -row[nz]))
        keep = nz[orderv[:K]]
        newrow = np.zeros_like(row)
        newrow[keep] = row[keep]
        sel_all[r] = newrow
    return sel_all


def kernel(**inputs):
    inputs = {k: np.asarray(v) for k, v in inputs.items()}
    x = inputs["x"]
    gate_w = inputs["gate_w"]
    sh1_w, sh1_b = inputs["sh1_w"], inputs["sh1_b"]
    sh2_w, sh2_b = inputs["sh2_w"], inputs["sh2_b"]
    e1_w, e1_b = inputs["e1_w"], inputs["e1_b"]
    e2_w, e2_b = inputs["e2_w"], inputs["e2_b"]

    xf = np.ascontiguousarray(x.reshape(T, D).astype(np.float32))
    xT = np.ascontiguousarray(xf.T)              # (D, T) fp32
    xT16 = xT.astype(BF16_NP)
    gwT = np.ascontiguousarray(gate_w.astype(np.float32).T)

    # ---- phase 1: gate ----
    maps1 = []
    for c in range(NCORES):
        blk = xf[c * TPC:(c + 1) * TPC]          # (512, 768)
        xg = np.ascontiguousarray(
            blk.reshape(MT, 128, KD, 128).transpose(0, 3, 2, 1))
        maps1.append({"xg": xg, "gwT": gwT})
    res1 = _run(_get_phase1(), maps1)

    sel_all = np.concatenate([res1[c]["sel"] for c in range(NCORES)], axis=0)
    sel_all = _fix_ties(sel_all)
    prob_colsum = np.sum(
        [res1[c]["colsums"][0, :E] for c in range(NCORES)], axis=0)

    # ---- host routing ----
    tok_idx, exp_idx = np.nonzero(sel_all > 0)   # token-major
    order = np.argsort(exp_idx, kind="stable")
    s_tok = tok_idx[order]
    s_exp = exp_idx[order]
    s_sc = sel_all[tok_idx, exp_idx][order].astype(np.float32)
    counts = np.bincount(s_exp, minlength=E)
    offs_e = np.concatenate([[0], np.cumsum(counts)])
    pos = np.arange(len(s_exp)) - offs_e[s_exp]

    # serpentine assignment of experts (sorted by count desc) to cores/slots
    sort_e = np.argsort(-counts, kind="stable")
    core_of_e = np.empty(E, np.int64)
    slot_of_e = np.empty(E, np.int64)
    for i, e in enumerate(sort_e):
        rnd = i // NCORES
        j = i % NCORES
        core_of_e[e] = j if rnd % 2 == 0 else NCORES - 1 - j
        slot_of_e[e] = 1 + rnd                   # slot 0 is the shared expert
    # static slot widths: max count over the cores at each slot, 16-aligned
    widths = np.zeros(NSLOT, np.int64)
    widths[0] = TPC
    for s in range(1, NSLOT):
        es = np.nonzero(slot_of_e == s)[0]
        widths[s] = max(16, -(-int(counts[es].max()) // 8) * 8)
    offs_s = np.concatenate([[0], np.cumsum(widths)]).astype(int)
    SLOTS = int(offs_s[-1])

    core_of = core_of_e[s_exp]
    local_slot = offs_s[slot_of_e[s_exp]] + pos

    # per-slot weights: slot 0 shared, slots 1.. routed by assignment
    f32 = np.float32
    def lay_w1(w):          # (Hd, D) -> [KH, 128, KD*128] lhsT blocks
        a = w.reshape(KH, 128, KD, 128).transpose(0, 3, 2, 1)
        return np.ascontiguousarray(a).reshape(KH, 128, KD * 128)

    def lay_w2(w):          # (D, Hd) -> [KD, 128, KH*128] lhsT blocks
        a = w.reshape(KD, 128, KH, 128).transpose(0, 3, 2, 1)
        return np.ascontiguousarray(a).reshape(KD, 128, KH * 128)

    w1_all = np.empty((NSLOT, NCORES, KH, 128, KD * 128), BF16_NP)
    w2_all = np.empty((NSLOT, NCORES, KD, 128, KH * 128), BF16_NP)
    b1_all = np.empty((NSLOT, NCORES, 128, KH), f32)
    b2_all = np.empty((NSLOT, NCORES, 128, KD), f32)
    w1_all[0] = lay_w1(sh1_w.astype(BF16_NP))
    w2_all[0] = lay_w2(sh2_w.astype(BF16_NP))
    b1_all[0] = sh1_b.astype(f32).reshape(KH, 128).T
    b2_all[0] = sh2_b.astype(f32).reshape(KD, 128).T
    for e in range(E):
        c, s = core_of_e[e], slot_of_e[e]
        w1_all[s, c] = lay_w1(e1_w[e].astype(BF16_NP))
        w2_all[s, c] = lay_w2(e2_w[e].astype(BF16_NP))
        b1_all[s, c] = e1_b[e].astype(f32).reshape(KH, 128).T
        b2_all[s, c] = e2_b[e].astype(f32).reshape(KD, 128).T

    maps2 = []
    for c in range(NCORES):
        mcore = core_of == c
        tokmap = np.zeros(SLOTS, np.int64)
        scr = np.zeros(SLOTS, np.float32)
        tokmap[local_slot[mcore]] = s_tok[mcore]
        scr[local_slot[mcore]] = s_sc[mcore]
        # shared slot: this core's own tokens, score 1
        tokmap[0:TPC] = np.arange(c * TPC, (c + 1) * TPC)
        scr[0:TPC] = 1.0
        maps2.append({
            "xdT": np.ascontiguousarray(xT16[:, tokmap]),
            "w1T": np.ascontiguousarray(w1_all[:, c]),
            "w2T": np.ascontiguousarray(w2_all[:, c]),
            "b1": np.ascontiguousarray(b1_all[:, c]),
            "b2": np.ascontiguousarray(b2_all[:, c]),
            "scores": scr[None, :],
        })

    res2 = _run(_get_phase2(widths), maps2)

    # ---- combine ----
    yd_all = np.concatenate(
        [res2[c]["ydT"].T for c in range(NCORES)], axis=0)  # (8*SLOTS, D)
    shared_out = np.concatenate(
        [yd_all[c * SLOTS:c * SLOTS + TPC] for c in range(NCORES)], axis=0)
    gslot = core_of * SLOTS + local_slot                    # expert-major
    inv_order = np.argsort(order, kind="stable")            # token-major view
    vals = yd_all[gslot[inv_order]]
    routed = vals.reshape(T, K, D).sum(axis=1, dtype=np.float32)

    out = (shared_out + routed).reshape(B, N, D).astype(np.float32)
    expert_prob = (prob_colsum / np.float32(T)).astype(np.float32)
    expert_freq = (counts / np.float32(T * K)).astype(np.float32)
    balance_loss = np.float32(0.01) * np.sum(
        expert_freq * expert_prob, dtype=np.float32)
    return out, np.float32(balance_loss), expert_freq, expert_prob
